# revision 40
# baseline (speedup 1.0000x reference)
"""DepthAwareGAT (3x GATConv + edge-encoder MLP) on 8 Trainium2 NeuronCores.

Sharding: edges sorted by destination; 8 contiguous dst ranges (one per core).
Nodes are assigned round-robin to 4 quarters (= gather chunks) and bin-packed
into 128-node dst tiles balancing per-(tile, chunk) edge counts, so nearly all
(tile, chunk) pairs need 4 edge blocks instead of 5 (pad ~6%).

Per layer: each core projects its node shard into a table T=[h|a_s|a_d] (bf16,
512B rows, head features c-major so the per-edge alpha broadcast multiply hits
the DVE 2x fast path); the table is AllGathered in 4 quarter-sized collectives
that fire as each quarter's projection tiles finish. Edge-parallel attention:
rows gathered by src via GPSIMD dma_gather (int16 indices over 4 table chunks
on 4 SWDGE queues), a_d[dst] expanded via fp8 one-hot S^T matmuls, and segment
softmax + weighted aggregation fused into one matmul per 128-edge block against
the block's one-hot S (fp8, host-precomputed; both orientations interleaved in
the `ss` array). The edge-encoder MLP is emitted interleaved with layer-1
attention so the in-order engine streams overlap it with the gathers. Block
structure is padded to the per-(dst-tile, chunk) max across cores so one SPMD
program serves all 8 cores.
"""
import os
import sys
import numpy as np
import ml_dtypes

sys.path.insert(0, "/opt/trn_rl_repo")
sys.path.insert(0, "/opt/trn_rl_repo/concourse")

N = 100000
E = 1600000
FIN = 64
HID = 32
H = 4
NC5 = 5
EF = 18
HC = H * HID          # 128
P = 128
NCH = 4               # gather-table chunks (int16 index range)
SGT = int(os.environ.get("GAT_SGT", "2"))  # dst-tiles per gather call group
NCORE = 8
IND_CH = frozenset(
    int(c) for c in os.environ.get("GAT_IND_CH", "").split(",") if c != "")
ROW = 256             # bf16 elems per T row, layers 1/2: [h128|as4|ad4|pad]
ROW3 = 128            # layer-3 rows: [h5|as1|ad1|pad]
BF16 = ml_dtypes.bfloat16
FP8 = ml_dtypes.float8_e4m3

# c-major head permutation: new col c*H+h <- old col h*HID+c
P_CM = np.arange(HC).reshape(H, HID).T.ravel()


def _blockdiag(att, heads, C):
    M = np.zeros((heads * C, heads), np.float32)
    for h in range(heads):
        M[h * C:(h + 1) * C, h] = att[h]
    return M


def _prep(inputs):
    src = np.asarray(inputs["edge_index"][0]).astype(np.int64)
    dst = np.asarray(inputs["edge_index"][1]).astype(np.int64)
    ea = np.asarray(inputs["edge_attr"])
    x = np.asarray(inputs["x"])

    order = np.argsort(dst, kind="stable")
    dsts = dst[order]
    pos = [0]
    for k in range(1, NCORE):
        p = k * E // NCORE
        while p < E and dsts[p] == dsts[p - 1]:
            p += 1
        pos.append(p)
    pos.append(E)
    n_lo = [0]
    for k in range(1, NCORE):
        n_lo.append(int(dsts[pos[k]]) if pos[k] < E else N)
    n_lo.append(N)
    n_lo = np.array(n_lo, np.int64)
    sizes = n_lo[1:] - n_lo[:-1]
    # NSH multiple of NCH*P so each of the NCH=4 quarters is whole tiles
    QUANT = NCH * P
    NSH = int(np.ceil(sizes.max() / QUANT) * QUANT)
    QSH = NSH // NCH
    CH = NCORE * QSH
    assert CH <= 32767, f"chunk rows {CH} exceed int16 range"
    NTILE = NSH // P
    NSG = NTILE // SGT
    QT = NTILE // NCH            # tiles per quarter

    core_of = np.searchsorted(n_lo[1:], np.arange(N), side="right")
    local_old = np.arange(N) - n_lo[core_of]
    quarter = local_old % NCH    # balanced chunk membership (pre-binpack)

    # in-degree per (node, src-chunk) for dst-tile bin-packing
    deg4 = np.zeros((N, NCH), np.int64)
    np.add.at(deg4, (dst, quarter[src]), 1)

    # bin-pack nodes into tiles within their (core, quarter), balancing
    # per-(tile, chunk) edge counts so most btc land at 4 blocks not 5
    new_local = np.zeros(N, np.int64)
    for k in range(NCORE):
        core_nodes = np.arange(n_lo[k], n_lo[k + 1])
        for q in range(NCH):
            nodes = core_nodes[quarter[core_nodes] == q]
            if len(nodes) == 0:
                continue
            d4 = deg4[nodes].astype(np.float64)
            ordn = np.argsort(-d4.max(axis=1), kind="stable")
            cnt = np.zeros((QT, NCH))
            nct = np.zeros(QT, np.int64)
            tile_in_q = np.zeros(len(nodes), np.int64)
            lane = np.zeros(len(nodes), np.int64)
            for i in ordn:
                d = d4[i]
                score = np.max(cnt + d[None, :], axis=1) + 0.001 * nct
                score[nct >= P] = np.inf
                t = int(np.argmin(score))
                tile_in_q[i] = t
                lane[i] = nct[t]
                cnt[t] += d
                nct[t] += 1
            new_local[nodes] = (q * QT + tile_in_q) * P + lane
    # table row within chunk: core-major quarters
    rowid = core_of * QSH + (new_local - quarter * QSH)

    per_core = []
    counts = np.zeros((NCORE, NTILE, NCH), np.int64)
    for k in range(NCORE):
        ek = order[pos[k]:pos[k + 1]]
        sk = src[ek]
        nl = new_local[dst[ek]]
        chunk = quarter[sk]
        slocal = rowid[sk].astype(np.int16)
        tile_ = nl // P
        ld = (nl % P).astype(np.uint8)
        key = tile_ * NCH + chunk
        o2 = np.argsort(key, kind="stable")
        per_core.append((slocal[o2], ld[o2], ek[o2], key[o2]))
        counts[k] = np.bincount(key, minlength=NTILE * NCH).reshape(NTILE, NCH)

    btc = np.ceil(counts.max(axis=0) / P).astype(np.int64)

    boff = np.zeros((NTILE, NCH), np.int64)
    calls, sginfo = [], []
    cur = 0
    for sg in range(NSG):
        sgb0 = cur
        cc = []
        for c in range(NCH):
            cb0 = cur
            for t in range(sg * SGT, (sg + 1) * SGT):
                boff[t, c] = cur
                cur += btc[t, c]
            cc.append((cb0, cur))
        calls.append(cc)
        sginfo.append((sgb0, cur - sgb0))
    calls = [[(int(a), int(b)) for a, b in cc] for cc in calls]
    sginfo = [(int(a), int(b)) for a, b in sginfo]
    TOTBLK = int(cur)
    TOTE = TOTBLK * P
    lb0 = np.cumsum(
        np.concatenate([np.zeros((NTILE, 1), np.int64), btc[:, :-1]], 1), 1)
    nblk = btc.sum(axis=1)

    in_maps_core = []
    eaN = np.concatenate([ea.astype(np.float32), np.zeros((1, EF), np.float32)])
    for k in range(NCORE):
        slocal, ld, eidx, key = per_core[k]
        cnt = counts[k]
        run_start = np.cumsum(np.concatenate([[0], cnt.ravel()[:-1]])).reshape(
            NTILE, NCH)
        cidx = np.zeros(TOTE, np.int16)
        cld = np.full(TOTE, 255, np.uint8)
        ceix = np.full(TOTE, E, np.int64)
        for t in range(NTILE):
            for c in range(NCH):
                n = int(cnt[t, c])
                if n == 0:
                    continue
                a = int(run_start[t, c])
                base = int(boff[t, c]) * P
                cidx[base:base + n] = slocal[a:a + n]
                cld[base:base + n] = ld[a:a + n]
                ceix[base:base + n] = eidx[a:a + n]
        gidx = np.zeros((16, TOTE // 16), np.int16)
        for sg in range(NSG):
            for c in range(NCH):
                cb0, cb1 = calls[sg][c]
                if cb1 == cb0:
                    continue
                a = cidx[cb0 * P:cb1 * P]
                gidx[:, cb0 * 8:cb1 * 8] = a.reshape(-1, 16).T
        gidx = np.tile(gidx, (8, 1))
        # int32 indices in partition-major order for the HWDGE indirect path
        gidx32 = np.ascontiguousarray(
            cidx.astype(np.int32).reshape(TOTBLK, P).T)
        ldm = cld.reshape(TOTBLK, P)
        # ss: per block [st (dstlane-partition) | se (edge-partition)] fp8
        oh = (ldm[:, None, :] == np.arange(P, dtype=np.uint8)[None, :, None])
        ss_np = np.empty((TOTBLK, P, 2 * P), np.bool_)
        ss_np[:, :, :P] = oh                       # st[j, e]
        ss_np[:, :, P:] = oh.transpose(0, 2, 1)    # se[e, j]
        ss = np.ascontiguousarray(
            ss_np.transpose(1, 0, 2).reshape(P, TOTBLK * 2 * P)).astype(FP8)
        eaT = np.ascontiguousarray(eaN[ceix].T).astype(BF16)
        xT = np.zeros((FIN, NSH), BF16)
        ids = np.arange(n_lo[k], n_lo[k + 1])
        xT[:, new_local[ids]] = x[ids].T.astype(BF16)
        in_maps_core.append(dict(gidx=gidx, gidx32=gidx32, ss=ss, eaT=eaT,
                                 xT=xT))

    g = lambda n: np.asarray(inputs[n], np.float32)
    Mcat = np.concatenate([
        g("we1") @ _blockdiag(g("ae1"), H, HID),
        g("we2") @ _blockdiag(g("ae2"), H, HID),
        g("we3") @ _blockdiag(g("ae3"), 1, NC5)], axis=1)

    def wext(w, a_s, a_d, heads, C, cm):
        h = w[:, :]
        out = np.concatenate(
            [h, w @ _blockdiag(a_s, heads, C), w @ _blockdiag(a_d, heads, C)],
            axis=1)
        if cm:
            out[:, :heads * C] = out[:, :heads * C][:, P_CM]
        return out

    w1e = wext(g("w1"), g("as1"), g("ad1"), H, HID, True)
    # layers 2/3 consume c-major h: permute input rows
    w2e = wext(g("w2"), g("as2"), g("ad2"), H, HID, True)[P_CM, :]
    w3e = wext(g("w3"), g("as3"), g("ad3"), 1, NC5, False)[P_CM, :]
    shared = dict(
        w1ext=w1e.astype(BF16),
        w2ext=w2e.astype(BF16),
        w3ext=w3e.astype(BF16),
        ew1=g("ew1").astype(BF16),
        eb1col=np.ascontiguousarray(g("eb1").reshape(HID, 1)),
        w2f=(g("ew2") @ Mcat).astype(BF16),
        cfrow_rep=np.tile(
            np.ascontiguousarray((g("eb2") @ Mcat).reshape(1, 9)), (P, 1)
        ).astype(BF16),
        brep1=np.tile(g("b1")[None, P_CM], (P, 1)),
        brep2=np.tile(g("b2")[None, P_CM], (P, 1)),
        b3rep=np.tile(g("b3")[None, :], (P, 1)),
        al02=np.full((P, 1), 0.2, np.float32),
        idn128=np.eye(P, dtype=np.float32).astype(BF16),
    )
    struct = dict(NSH=NSH, NTILE=NTILE, NSG=NSG, CH=CH, QSH=QSH, QT=QT,
                  TOTBLK=TOTBLK, TOTE=TOTE,
                  btc=btc, boff=boff, lb0=lb0, nblk=nblk, calls=calls,
                  sginfo=sginfo, n_lo=n_lo, new_local=new_local,
                  MAXB=int(btc.max()),
                  MAXNBLK=int(nblk.max()),
                  MAXCALL=max(cb1 - cb0 for cc in calls for cb0, cb1 in cc),
                  MAXSGB=max(sb for _, sb in sginfo))
    return in_maps_core, shared, struct


def _build(s, n_layers=3, dbg_layer=-1):
    import concourse.bass as bass
    import concourse.bacc as bacc
    import concourse.mybir as mybir
    import concourse.tile as tile

    A = mybir.ActivationFunctionType
    OP = mybir.AluOpType
    FP32 = mybir.dt.float32
    BF = mybir.dt.bfloat16
    F8 = mybir.dt.float8e4
    I16 = mybir.dt.int16
    I32 = mybir.dt.int32

    NSH, NTILE, NSG, CH = s["NSH"], s["NTILE"], s["NSG"], s["CH"]
    QSH, QT = s["QSH"], s["QT"]
    TOTBLK, TOTE = s["TOTBLK"], s["TOTE"]
    btc, boff, lb0, nblk = s["btc"], s["boff"], s["lb0"], s["nblk"]
    calls, sginfo = s["calls"], s["sginfo"]
    MAXB, MAXNBLK = s["MAXB"], s["MAXNBLK"]
    MAXCALL, MAXSGB = s["MAXCALL"], s["MAXSGB"]

    nc = bacc.Bacc("TRN2", target_bir_lowering=False, debug=False,
                   enable_asserts=False, num_devices=NCORE, num_swdge_queues=4)

    def dt_in(name, shape, dt):
        return nc.dram_tensor(name, list(shape), dt, kind="ExternalInput").ap()

    gidx_d = dt_in("gidx", (P, TOTE // 16), I16)
    gidx32_d = dt_in("gidx32", (P, TOTBLK), I32)
    ss_d = dt_in("ss", (P, TOTBLK * 2 * P), F8)
    eaT_d = dt_in("eaT", (EF, TOTE), BF)
    xT_d = dt_in("xT", (FIN, NSH), BF)
    w1ext_d = dt_in("w1ext", (FIN, 136), BF)
    w2ext_d = dt_in("w2ext", (HC, 136), BF)
    w3ext_d = dt_in("w3ext", (HC, 7), BF)
    ew1_d = dt_in("ew1", (EF, HID), BF)
    eb1col_d = dt_in("eb1col", (HID, 1), FP32)
    w2f_d = dt_in("w2f", (HID, 9), BF)
    cfrow_rep_d = dt_in("cfrow_rep", (P, 9), BF)
    brep1_d = dt_in("brep1", (P, HC), FP32)
    brep2_d = dt_in("brep2", (P, HC), FP32)
    b3rep_d = dt_in("b3rep", (P, NC5), FP32)
    al02_d = dt_in("al02", (P, 1), FP32)
    idn128_d = dt_in("idn128", (P, P), BF)

    out_d = nc.dram_tensor("out", [NSH, NC5], FP32, kind="ExternalOutput").ap()
    dbg_d = None
    if dbg_layer >= 0:
        dbg_d = nc.dram_tensor("dbg_ht", [P, NSH], FP32,
                               kind="ExternalOutput").ap()

    def mk(base_ap, extra_off, dims):
        return bass.AP(base_ap.tensor, base_ap.offset + extra_off,
                       [base_ap.ap[0]] + dims)

    with tile.TileContext(nc) as tc:
        with tc.tile_pool(name="const", bufs=1) as cst, \
             tc.tile_pool(name="big", bufs=1) as big, \
             tc.tile_pool(name="dram", bufs=1, space="DRAM") as dr:

            def ld_const(ap, shape, dt, nm):
                t = cst.tile(list(shape), dt, name=nm, tag=nm)
                nc.sync.dma_start(out=t[:], in_=ap[:, :])
                return t

            w1ext = ld_const(w1ext_d, (FIN, 136), BF, "w1ext")
            w2ext = ld_const(w2ext_d, (HC, 136), BF, "w2ext")
            w3ext = ld_const(w3ext_d, (HC, 7), BF, "w3ext")
            ew1 = ld_const(ew1_d, (EF, HID), BF, "ew1")
            eb1col = ld_const(eb1col_d, (HID, 1), FP32, "eb1col")
            w2f = ld_const(w2f_d, (HID, 9), BF, "w2f")
            cfrow_rep = ld_const(cfrow_rep_d, (P, 9), BF, "cfrow_rep")
            brep1 = ld_const(brep1_d, (P, HC), FP32, "brep1")
            brep2 = ld_const(brep2_d, (P, HC), FP32, "brep2")
            b3rep = ld_const(b3rep_d, (P, NC5), FP32, "b3rep")
            al02 = ld_const(al02_d, (P, 1), FP32, "al02")
            idn128 = ld_const(idn128_d, (P, P), BF, "idn128")
            # small epilogue constants built on-chip
            epsH = cst.tile([P, H], FP32, name="epsH", tag="epsH")
            nc.vector.memset(epsH[:], 1e-16)
            ones_hc = cst.tile([P, HC], FP32, name="ones_hc", tag="ones_hc")
            nc.vector.memset(ones_hc[:], 1.0)
            zero_hc = cst.tile([P, HC], FP32, name="zero_hc", tag="zero_hc")
            nc.vector.memset(zero_hc[:], 0.0)
            eps1 = cst.tile([P, 1], FP32, name="eps1", tag="eps1")
            nc.vector.memset(eps1[:], 1e-16)
            zero1 = cst.tile([P, 1], FP32, name="zero1", tag="zero1")
            nc.vector.memset(zero1[:], 0.0)

            ht = big.tile([P, NSH], BF)
            adsb = [big.tile([P, NTILE * H], BF, name=f"adsb{i}", tag=f"adsb{i}")
                    for i in range(3)]
            AEC = dr.tile([P, TOTBLK * 9], BF, name="aecd")

            Tsh = [[dr.tile([QSH, ROW if l < 2 else ROW3], BF,
                            name=f"tsh{l}q{q}") for q in range(NCH)]
                   for l in range(3)]
            Tf = [[dr.tile([CH, ROW if l < 2 else ROW3], BF,
                           name=f"tf{l}q{q}", addr_space="Shared")
                   for q in range(NCH)] for l in range(3)]

            xt_cm = tc.tile_pool(name="xtp", bufs=1)
            xt_pool = xt_cm.__enter__()
            xt = xt_pool.tile([FIN, NSH], BF, name="xt")
            nc.sync.dma_start(out=xt[:], in_=xT_d[:, :])

            def projection(lay, pps, stg_p):
                K = FIN if lay == 0 else HC
                lhs = xt if lay == 0 else ht
                wx = (w1ext, w2ext, w3ext)[lay]
                ncol = 7 if lay == 2 else 136
                rw = ROW3 if lay == 2 else ROW
                adw = 1 if lay == 2 else H
                adoff = 6 if lay == 2 else 132
                for tp in range(NTILE):
                    pp = pps.tile([P, 136], FP32, space="PSUM", tag="proj",
                                  name="proj")
                    nc.tensor.matmul(pp[:, :ncol],
                                     lhsT=lhs[:K, tp * P:(tp + 1) * P],
                                     rhs=wx[:], start=True, stop=True)
                    st_t = stg_p.tile([P, ROW], BF, tag="tstg", name="tstg")
                    nc.vector.tensor_copy(out=st_t[:, :ncol], in_=pp[:, :ncol])
                    nc.vector.tensor_copy(
                        out=adsb[lay][:, tp * adw:(tp + 1) * adw],
                        in_=pp[:, adoff:adoff + adw])
                    q, tq = tp // QT, tp % QT
                    nc.sync.dma_start(
                        out=Tsh[lay][q][tq * P:(tq + 1) * P, :],
                        in_=st_t[:, :rw])
                    if tq == QT - 1:
                        nc.gpsimd.collective_compute(
                            "AllGather", OP.bypass,
                            replica_groups=[list(range(NCORE))],
                            ins=[Tsh[lay][q].opt()], outs=[Tf[lay][q].opt()])

            # ---------------- layer-1 projection (before encoder so the
            # AllGather + gathers start as early as possible) ---------------
            with tc.tile_pool(name="p0ps", bufs=2, space="PSUM") as p0ps, \
                 tc.tile_pool(name="p0stg", bufs=3) as p0stg:
                projection(0, p0ps, p0stg)
            xt_cm.__exit__(None, None, None)

            # ---------------- layers (encoder interleaved into L1) --------
            with tc.tile_pool(name="mps", bufs=2, space="PSUM") as pps, \
                 tc.tile_pool(name="ade_ps", bufs=1, space="PSUM") as pade, \
                 tc.tile_pool(name="agg_ps", bufs=2, space="PSUM") as pagg, \
                 tc.tile_pool(name="tr_ps", bufs=1, space="PSUM") as ptr, \
                 tc.tile_pool(name="enc_sb", bufs=4) as esb, \
                 tc.tile_pool(name="enc_ps", bufs=1, space="PSUM") as eps, \
                 tc.tile_pool(name="enc_ps2", bufs=1, space="PSUM") as eps2, \
                 tc.tile_pool(name="stgp", bufs=3) as stg_p, \
                 tc.tile_pool(name="gp", bufs=10 if SGT == 1 else (5 if SGT <= 2 else 2)) as gp, \
                 tc.tile_pool(name="stp", bufs=2) as stp, \
                 tc.tile_pool(name="zp", bufs=4) as zp, \
                 tc.tile_pool(name="ep", bufs=4) as ep, \
                 tc.tile_pool(name="ip", bufs=10) as ip, \
                 tc.tile_pool(name="aep", bufs=3) as aep:

                EG = 16
                enc_next = [0]

                def emit_enc(ngroups):
                    for _ in range(ngroups):
                        eg0 = enc_next[0]
                        if eg0 >= TOTBLK:
                            return
                        enc_next[0] = eg0 + EG
                        nb = min(EG, TOTBLK - eg0)
                        ne = nb * P
                        ea_t = esb.tile([EF, EG * P], BF, tag="ea", name="ea")
                        nc.sync.dma_start(out=ea_t[:, :ne],
                                          in_=eaT_d[:, eg0 * P:eg0 * P + ne])
                        aest = esb.tile([P, EG * 9], BF, tag="aest", name="aest")
                        for q0 in range(0, ne, 512):
                            qn = min(512, ne - q0)
                            nsub = qn // P
                            hidp = eps.tile([HID, 512], FP32, space="PSUM",
                                            tag="hid", name="hid")
                            nc.tensor.matmul(hidp[:, :qn], lhsT=ew1[:],
                                             rhs=ea_t[:, q0:q0 + qn],
                                             start=True, stop=True)
                            hids = esb.tile([HID, 512], BF, tag="hids",
                                            name="hids")
                            nc.scalar.activation(hids[:, :qn], hidp[:, :qn],
                                                 A.Relu, bias=eb1col[:],
                                                 scale=1.0)
                            pae = eps2.tile([P, 36], FP32, space="PSUM",
                                            tag="pae", name="pae")
                            for sb_ in range(nsub):
                                sl = pae[:, sb_ * 9:sb_ * 9 + 9]
                                nc.tensor.matmul(
                                    sl, lhsT=hids[:, sb_ * P:(sb_ + 1) * P],
                                    rhs=w2f[:], start=True, stop=True)
                            col = (q0 // P) * 9
                            nc.vector.tensor_tensor(
                                out=aest[:, col:col + nsub * 9],
                                in0=pae[:, :nsub * 9],
                                in1=mk(cfrow_rep[:], 0, [[0, nsub], [1, 9]]),
                                op=OP.add)
                        nc.sync.dma_start(out=AEC[:, eg0 * 9:(eg0 + nb) * 9],
                                          in_=aest[:, :nb * 9])

                def attention(lay, pre=None):
                    rw = ROW3 if lay == 2 else ROW
                    vw = 6 if lay == 2 else 132
                    aw = 1 if lay == 2 else H
                    acol = NC5 if lay == 2 else HC
                    CC = NC5 if lay == 2 else HID  # features per head
                    aecol = (0, 4, 8)[lay]
                    brep = (brep1, brep2, None)[lay]
                    for sg in range(NSG):
                        if pre is not None:
                            pre()
                        sgb0, sgblk = sginfo[sg]
                        if sgblk == 0:
                            continue
                        aec_t = aep.tile([P, MAXSGB * 9], BF, tag="aec",
                                         name="aec")
                        nc.sync.dma_start(out=aec_t[:, :sgblk * 9],
                                          in_=AEC[:, sgb0 * 9:(sgb0 + sgblk) * 9])
                        ss_sg = stp.tile([P, MAXSGB * 2 * P], F8, tag="ss",
                                         name="ss")
                        nc.scalar.dma_start(
                            out=ss_sg[:, :sgblk * 2 * P],
                            in_=ss_d[:, sgb0 * 2 * P:(sgb0 + sgblk) * 2 * P])
                        g_t = {}
                        for c in range(NCH):
                            cb0, cb1 = calls[sg][c]
                            nn = cb1 - cb0
                            if nn == 0:
                                continue
                            gt = gp.tile([P, MAXCALL, rw], BF, tag=f"g{c}")
                            if c in IND_CH:
                                it32 = ip.tile([P, MAXCALL], I32, tag="idx32",
                                               name="idx32")
                                nc.sync.dma_start(out=it32[:, :nn],
                                                  in_=gidx32_d[:, cb0:cb1])
                                nc.gpsimd.indirect_dma_start(
                                    out=gt[:, :nn, :], out_offset=None,
                                    in_=Tf[lay][c][:, :],
                                    in_offset=bass.IndirectOffsetOnAxis(
                                        ap=it32[:, :nn], axis=0))
                            else:
                                it = ip.tile([P, MAXCALL * 8], I16, tag="idx",
                                             name="idx")
                                nc.sync.dma_start(out=it[:, :nn * 8],
                                                  in_=gidx_d[:, cb0 * 8:cb1 * 8])
                                nc.gpsimd.dma_gather(
                                    out_ap=gt[:, :nn, :],
                                    in_ap=Tf[lay][c][:, :],
                                    idxs_ap=it[:, :nn * 8],
                                    num_idxs=nn * P, num_idxs_reg=nn * P,
                                    elem_size=rw, single_packet=False,
                                    queue_num=c)
                            g_t[c] = gt
                        for t in range(sg * SGT, (sg + 1) * SGT):
                            nb = int(nblk[t])
                            aggp = pagg.tile([P, 132], FP32, space="PSUM",
                                             tag="agg")
                            if nb == 0:
                                nc.vector.memset(aggp[:, :vw], 0.0)
                            else:
                                adt_sl = adsb[lay][:, t * aw:(t + 1) * aw]
                                adep = pade.tile([P, MAXNBLK * H], FP32,
                                                 space="PSUM", tag="ade")
                                z1 = zp.tile([P, MAXNBLK * H], FP32, tag="z1",
                                             name="z1")
                                for c in range(NCH):
                                    b = int(btc[t, c])
                                    if b == 0:
                                        continue
                                    bo = int(boff[t, c])
                                    lb = int(lb0[t, c])
                                    s0 = bo - calls[sg][c][0]
                                    sb = bo - sgb0
                                    for bi in range(b):
                                        nc.tensor.matmul(
                                            adep[:, (lb + bi) * aw:
                                                 (lb + bi + 1) * aw],
                                            lhsT=ss_sg[:, (sb + bi) * 2 * P:
                                                       (sb + bi) * 2 * P + P],
                                            rhs=adt_sl,
                                            start=True, stop=True)
                                    gb = g_t[c][:]          # [P, MAXCALL, ROW]
                                    gstep = gb.ap[1][0]     # ROW stride
                                    z1sl = mk(z1[:], lb * aw,
                                              [[aw, b], [1, aw]])
                                    as_ap = mk(gb, s0 * gstep + acol,
                                               [[gstep, b], [1, aw]])
                                    ae_ap = mk(aec_t[:],
                                               ((bo - sgb0) * 9 + aecol),
                                               [[9, b], [1, aw]])
                                    nc.vector.tensor_tensor(
                                        out=z1sl, in0=as_ap, in1=ae_ap, op=OP.add)
                                zz = zp.tile([P, MAXNBLK * H], FP32, tag="zz",
                                             name="zz")
                                nc.vector.tensor_tensor(
                                    out=zz[:, :nb * aw], in0=z1[:, :nb * aw],
                                    in1=adep[:, :nb * aw], op=OP.add)
                                zpre = zp.tile([P, MAXNBLK * H], FP32,
                                               tag="zpre", name="zpre")
                                nc.scalar.activation(zpre[:, :nb * aw],
                                                     zz[:, :nb * aw], A.Prelu,
                                                     bias=0.0, scale=1.0,
                                                     alpha=al02[:])
                                mmi = 0
                                for c in range(NCH):
                                    b = int(btc[t, c])
                                    if b == 0:
                                        continue
                                    bo = int(boff[t, c])
                                    lb = int(lb0[t, c])
                                    s0 = bo - calls[sg][c][0]
                                    sb = bo - sgb0
                                    gb = g_t[c][:]
                                    gstep = gb.ap[1][0]
                                    ex_ap = mk(gb, s0 * gstep + acol,
                                               [[gstep, b], [1, aw]])
                                    nc.scalar.activation(
                                        ex_ap, mk(zpre[:], lb * aw,
                                                  [[aw, b], [1, aw]]),
                                        A.Exp, bias=0.0, scale=1.0)
                                    if lay < 2:
                                        # c-major: inner dim over H contiguous
                                        v_in = mk(gb, s0 * gstep,
                                                  [[gstep, b], [aw, CC],
                                                   [1, aw]])
                                        a_in = mk(gb, s0 * gstep + acol,
                                                  [[gstep, b], [0, CC],
                                                   [1, aw]])
                                    else:
                                        v_in = mk(gb, s0 * gstep,
                                                  [[gstep, b], [CC, aw],
                                                   [1, CC]])
                                        a_in = mk(gb, s0 * gstep + acol,
                                                  [[gstep, b], [1, aw],
                                                   [0, CC]])
                                    nc.vector.tensor_tensor(
                                        out=v_in, in0=v_in, in1=a_in, op=OP.mult)
                                    for bi in range(b):
                                        nc.tensor.matmul(
                                            aggp[:, :vw],
                                            lhsT=ss_sg[:, (sb + bi) * 2 * P + P:
                                                       (sb + bi + 1) * 2 * P],
                                            rhs=mk(gb, (s0 + bi) * gstep,
                                                   [[1, vw]]),
                                            start=(mmi == 0),
                                            stop=(mmi == nb - 1))
                                        mmi += 1
                            # epilogue: move [agg|den] to SBUF, free PSUM fast
                            agg_s = ep.tile([P, 132], FP32, tag="aggs",
                                            name="aggs")
                            nc.vector.tensor_copy(out=agg_s[:, :vw],
                                                  in_=aggp[:, :vw])
                            if lay < 2:
                                t1 = ep.tile([P, H], FP32, tag="t1", name="t1")
                                nc.vector.tensor_tensor(
                                    out=t1[:], in0=agg_s[:, HC:HC + H],
                                    in1=epsH[:], op=OP.add)
                                rden = ep.tile([P, H], FP32, tag="rden",
                                               name="rden")
                                nc.vector.reciprocal(out=rden[:], in_=t1[:])
                                xn = ep.tile([P, HC], FP32, tag="xn", name="xn")
                                # c-major: den broadcast inner-contiguous
                                nc.vector.tensor_tensor(
                                    out=xn[:], in0=agg_s[:, :HC],
                                    in1=mk(rden[:], 0, [[0, HID], [1, H]]),
                                    op=OP.mult)
                                xb = ep.tile([P, HC], FP32, tag="xb", name="xb")
                                nc.vector.tensor_tensor(out=xb[:], in0=xn[:],
                                                        in1=brep[:], op=OP.add)
                                e1 = ep.tile([P, HC], FP32, tag="e1", name="e1")
                                nc.scalar.activation(e1[:], xb[:], A.Exp,
                                                     bias=0.0, scale=1.0)
                                em1 = ep.tile([P, HC], FP32, tag="em1",
                                              name="em1")
                                nc.vector.tensor_tensor(out=em1[:], in0=e1[:],
                                                        in1=ones_hc[:],
                                                        op=OP.subtract)
                                t2 = ep.tile([P, HC], FP32, tag="t2", name="t2")
                                nc.vector.tensor_tensor(out=t2[:], in0=em1[:],
                                                        in1=zero_hc[:],
                                                        op=OP.min)
                                r1 = ep.tile([P, HC], FP32, tag="r1", name="r1")
                                nc.scalar.activation(r1[:], xb[:], A.Relu,
                                                     bias=0.0, scale=1.0)
                                hn = ep.tile([P, HC], BF, tag="hn", name="hn")
                                nc.vector.tensor_tensor(out=hn[:], in0=t2[:],
                                                        in1=r1[:], op=OP.add)
                                htp = ptr.tile([P, P], BF, space="PSUM",
                                               tag="htp")
                                nc.tensor.transpose(out=htp[:], in_=hn[:],
                                                    identity=idn128[:])
                                nc.vector.tensor_copy(
                                    out=ht[:, t * P:(t + 1) * P], in_=htp[:])
                            else:
                                t1 = ep.tile([P, 1], FP32, tag="t1", name="t1")
                                nc.vector.tensor_tensor(
                                    out=t1[:], in0=agg_s[:, NC5:NC5 + 1],
                                    in1=eps1[:], op=OP.add)
                                rden = ep.tile([P, 1], FP32, tag="rden",
                                               name="rden")
                                nc.vector.reciprocal(out=rden[:], in_=t1[:])
                                x5 = ep.tile([P, NC5], FP32, tag="xn", name="xn")
                                nc.vector.tensor_tensor(
                                    out=x5[:], in0=agg_s[:, :NC5],
                                    in1=mk(rden[:], 0, [[0, NC5]]),
                                    op=OP.mult)
                                xb5 = ep.tile([P, NC5], FP32, tag="xb",
                                              name="xb")
                                nc.vector.tensor_tensor(out=xb5[:], in0=x5[:],
                                                        in1=b3rep[:], op=OP.add)
                                m1 = ep.tile([P, 1], FP32, tag="m1", name="m1")
                                nc.vector.reduce_max(out=m1[:], in_=xb5[:],
                                                     axis=mybir.AxisListType.X)
                                negm = ep.tile([P, 1], FP32, tag="negm",
                                               name="negm")
                                nc.vector.tensor_tensor(
                                    out=negm[:], in0=zero1[:], in1=m1[:],
                                    op=OP.subtract)
                                e5 = ep.tile([P, NC5], FP32, tag="e1", name="e1")
                                nc.scalar.activation(e5[:], xb5[:], A.Exp,
                                                     bias=negm[:], scale=1.0)
                                ssum = ep.tile([P, 1], FP32, tag="ssum",
                                               name="ssum")
                                nc.vector.reduce_sum(out=ssum[:], in_=e5[:],
                                                     axis=mybir.AxisListType.X)
                                lns = ep.tile([P, 1], FP32, tag="lns",
                                              name="lns")
                                nc.scalar.activation(lns[:], ssum[:], A.Ln,
                                                     bias=0.0, scale=1.0)
                                mls = ep.tile([P, 1], FP32, tag="mls",
                                              name="mls")
                                nc.vector.tensor_tensor(out=mls[:], in0=m1[:],
                                                        in1=lns[:], op=OP.add)
                                o5 = ep.tile([P, NC5], FP32, tag="o5",
                                             name="o5")
                                nc.vector.tensor_tensor(
                                    out=o5[:], in0=xb5[:],
                                    in1=mk(mls[:], 0, [[0, NC5]]),
                                    op=OP.subtract)
                                nc.sync.dma_start(
                                    out=out_d[t * P:(t + 1) * P, :], in_=o5[:])

                for lay in range(n_layers):
                    if lay > 0:
                        projection(lay, pps, stg_p)
                    attention(lay, pre=(lambda: emit_enc(3))
                              if lay == 0 else None)
                    if lay == 0:
                        emit_enc(TOTBLK)  # drain any leftover encoder groups
                    if dbg_layer == lay and dbg_d is not None and lay < 2:
                        nc.gpsimd.dma_start(out=dbg_d[:, :], in_=ht[:])
    nc.compile()
    return nc


def kernel(**inputs):
    from concourse import bass_utils
    in_maps_core, shared, struct = _prep(inputs)
    n_layers = int(os.environ.get("GAT_LAYERS", "3"))
    dbg_layer = int(os.environ.get("GAT_DEBUG_LAYER", "-1"))
    nc = _build(struct, n_layers=n_layers, dbg_layer=dbg_layer)
    in_maps = []
    for k in range(NCORE):
        m = dict(in_maps_core[k])
        m.update(shared)
        in_maps.append(m)
    trace = os.environ.get("GAT_TRACE", "0") == "1"
    res = bass_utils.run_bass_kernel_spmd(
        nc, in_maps, core_ids=list(range(NCORE)), trace=trace)
    kernel.last_result = res
    kernel.last_struct = struct
    n_lo = struct["n_lo"]
    nl = struct["new_local"]
    out = np.zeros((N, NC5), np.float32)
    for k in range(NCORE):
        ids = np.arange(n_lo[k], n_lo[k + 1])
        out[ids] = res.results[k]["out"][nl[ids]]
    return out


# revision 41
# speedup vs baseline: 1.0723x; 1.0723x over previous
"""DepthAwareGAT (3x GATConv + edge-encoder MLP) on 8 Trainium2 NeuronCores.

Sharding: edges sorted by destination; 8 contiguous dst ranges (one per core).
Nodes are assigned round-robin to 4 quarters (= gather chunks) and bin-packed
into 128-node dst tiles balancing per-(tile, chunk) edge counts, so nearly all
(tile, chunk) pairs need 4 edge blocks instead of 5 (pad ~6%).

Per layer: each core projects its node shard into a table T=[h|a_s|a_d] (bf16,
512B rows, head features c-major so the per-edge alpha broadcast multiply hits
the DVE 2x fast path); the table is AllGathered in 4 quarter-sized collectives
that fire as each quarter's projection tiles finish. Edge-parallel attention:
rows gathered by src via GPSIMD dma_gather (int16 indices over 4 table chunks
on 4 SWDGE queues), a_d[dst] expanded via fp8 one-hot S^T matmuls, and segment
softmax + weighted aggregation fused into one matmul per 128-edge block against
the block's one-hot S (fp8, host-precomputed; both orientations interleaved in
the `ss` array). The edge-encoder MLP is emitted interleaved with layer-1
attention so the in-order engine streams overlap it with the gathers. Block
structure is padded to the per-(dst-tile, chunk) max across cores so one SPMD
program serves all 8 cores.
"""
import os
import sys
import numpy as np
import ml_dtypes

sys.path.insert(0, "/opt/trn_rl_repo")
sys.path.insert(0, "/opt/trn_rl_repo/concourse")

N = 100000
E = 1600000
FIN = 64
HID = 32
H = 4
NC5 = 5
EF = 18
HC = H * HID          # 128
P = 128
NCH = 4               # gather-table chunks (int16 index range)
SGT = int(os.environ.get("GAT_SGT", "2"))  # dst-tiles per gather call group
NCORE = 8
IND_CH = frozenset(
    int(c) for c in os.environ.get("GAT_IND_CH", "").split(",") if c != "")
ROW = 256             # bf16 elems per T row, layers 1/2: [h128|as4|ad4|pad]
ROW3 = 128            # layer-3 rows: [h5|as1|ad1|pad]
BF16 = ml_dtypes.bfloat16
FP8 = ml_dtypes.float8_e4m3

# c-major head permutation: new col c*H+h <- old col h*HID+c
P_CM = np.arange(HC).reshape(H, HID).T.ravel()


def _blockdiag(att, heads, C):
    M = np.zeros((heads * C, heads), np.float32)
    for h in range(heads):
        M[h * C:(h + 1) * C, h] = att[h]
    return M


def _prep(inputs):
    src = np.asarray(inputs["edge_index"][0]).astype(np.int64)
    dst = np.asarray(inputs["edge_index"][1]).astype(np.int64)
    ea = np.asarray(inputs["edge_attr"])
    x = np.asarray(inputs["x"])

    order = np.argsort(dst, kind="stable")
    dsts = dst[order]
    pos = [0]
    for k in range(1, NCORE):
        p = k * E // NCORE
        while p < E and dsts[p] == dsts[p - 1]:
            p += 1
        pos.append(p)
    pos.append(E)
    n_lo = [0]
    for k in range(1, NCORE):
        n_lo.append(int(dsts[pos[k]]) if pos[k] < E else N)
    n_lo.append(N)
    n_lo = np.array(n_lo, np.int64)
    sizes = n_lo[1:] - n_lo[:-1]
    # NSH multiple of NCH*P so each of the NCH=4 quarters is whole tiles
    QUANT = NCH * P
    NSH = int(np.ceil(sizes.max() / QUANT) * QUANT)
    QSH = NSH // NCH
    CH = NCORE * QSH
    assert CH <= 32767, f"chunk rows {CH} exceed int16 range"
    NTILE = NSH // P
    NSG = NTILE // SGT
    QT = NTILE // NCH            # tiles per quarter

    core_of = np.searchsorted(n_lo[1:], np.arange(N), side="right")
    local_old = np.arange(N) - n_lo[core_of]
    quarter = local_old % NCH    # balanced chunk membership (pre-binpack)

    # in-degree per (node, src-chunk) for dst-tile bin-packing
    deg4 = np.zeros((N, NCH), np.int64)
    np.add.at(deg4, (dst, quarter[src]), 1)

    # bin-pack nodes into tiles within their (core, quarter), balancing
    # per-(tile, chunk) edge counts so most btc land at 4 blocks not 5
    new_local = np.zeros(N, np.int64)
    for k in range(NCORE):
        core_nodes = np.arange(n_lo[k], n_lo[k + 1])
        for q in range(NCH):
            nodes = core_nodes[quarter[core_nodes] == q]
            if len(nodes) == 0:
                continue
            d4 = deg4[nodes].astype(np.float64)
            ordn = np.argsort(-d4.max(axis=1), kind="stable")
            cnt = np.zeros((QT, NCH))
            nct = np.zeros(QT, np.int64)
            tile_in_q = np.zeros(len(nodes), np.int64)
            lane = np.zeros(len(nodes), np.int64)
            for i in ordn:
                d = d4[i]
                score = np.max(cnt + d[None, :], axis=1) + 0.001 * nct
                score[nct >= P] = np.inf
                t = int(np.argmin(score))
                tile_in_q[i] = t
                lane[i] = nct[t]
                cnt[t] += d
                nct[t] += 1
            new_local[nodes] = (q * QT + tile_in_q) * P + lane
    # table row within chunk: core-major quarters
    rowid = core_of * QSH + (new_local - quarter * QSH)

    per_core = []
    counts = np.zeros((NCORE, NTILE, NCH), np.int64)
    for k in range(NCORE):
        ek = order[pos[k]:pos[k + 1]]
        sk = src[ek]
        nl = new_local[dst[ek]]
        chunk = quarter[sk]
        slocal = rowid[sk].astype(np.int16)
        tile_ = nl // P
        ld = (nl % P).astype(np.uint8)
        key = tile_ * NCH + chunk
        o2 = np.argsort(key, kind="stable")
        per_core.append((slocal[o2], ld[o2], ek[o2], key[o2]))
        counts[k] = np.bincount(key, minlength=NTILE * NCH).reshape(NTILE, NCH)

    btc = np.ceil(counts.max(axis=0) / P).astype(np.int64)

    boff = np.zeros((NTILE, NCH), np.int64)
    calls, sginfo = [], []
    cur = 0
    for sg in range(NSG):
        sgb0 = cur
        cc = []
        for c in range(NCH):
            cb0 = cur
            for t in range(sg * SGT, (sg + 1) * SGT):
                boff[t, c] = cur
                cur += btc[t, c]
            cc.append((cb0, cur))
        calls.append(cc)
        sginfo.append((sgb0, cur - sgb0))
    calls = [[(int(a), int(b)) for a, b in cc] for cc in calls]
    sginfo = [(int(a), int(b)) for a, b in sginfo]
    TOTBLK = int(cur)
    TOTE = TOTBLK * P
    lb0 = np.cumsum(
        np.concatenate([np.zeros((NTILE, 1), np.int64), btc[:, :-1]], 1), 1)
    nblk = btc.sum(axis=1)

    in_maps_core = []
    eaN = np.concatenate([ea.astype(np.float32), np.zeros((1, EF), np.float32)])
    for k in range(NCORE):
        slocal, ld, eidx, key = per_core[k]
        cnt = counts[k]
        run_start = np.cumsum(np.concatenate([[0], cnt.ravel()[:-1]])).reshape(
            NTILE, NCH)
        cidx = np.zeros(TOTE, np.int16)
        cld = np.full(TOTE, 255, np.uint8)
        ceix = np.full(TOTE, E, np.int64)
        for t in range(NTILE):
            for c in range(NCH):
                n = int(cnt[t, c])
                if n == 0:
                    continue
                a = int(run_start[t, c])
                base = int(boff[t, c]) * P
                cidx[base:base + n] = slocal[a:a + n]
                cld[base:base + n] = ld[a:a + n]
                ceix[base:base + n] = eidx[a:a + n]
        gidx = np.zeros((16, TOTE // 16), np.int16)
        for sg in range(NSG):
            for c in range(NCH):
                cb0, cb1 = calls[sg][c]
                if cb1 == cb0:
                    continue
                a = cidx[cb0 * P:cb1 * P]
                gidx[:, cb0 * 8:cb1 * 8] = a.reshape(-1, 16).T
        gidx = np.tile(gidx, (8, 1))
        # int32 indices in partition-major order for the HWDGE indirect path
        gidx32 = np.ascontiguousarray(
            cidx.astype(np.int32).reshape(TOTBLK, P).T)
        ldm = cld.reshape(TOTBLK, P)
        # ss: per block [st (dstlane-partition) | se (edge-partition)] fp8
        oh = (ldm[:, None, :] == np.arange(P, dtype=np.uint8)[None, :, None])
        ss_np = np.empty((TOTBLK, P, 2 * P), np.bool_)
        ss_np[:, :, :P] = oh                       # st[j, e]
        ss_np[:, :, P:] = oh.transpose(0, 2, 1)    # se[e, j]
        ss = np.ascontiguousarray(
            ss_np.transpose(1, 0, 2).reshape(P, TOTBLK * 2 * P)).astype(FP8)
        eaT = np.ascontiguousarray(eaN[ceix].T).astype(BF16)
        xT = np.zeros((FIN, NSH), BF16)
        ids = np.arange(n_lo[k], n_lo[k + 1])
        xT[:, new_local[ids]] = x[ids].T.astype(BF16)
        in_maps_core.append(dict(gidx=gidx, gidx32=gidx32, ss=ss, eaT=eaT,
                                 xT=xT))

    g = lambda n: np.asarray(inputs[n], np.float32)
    Mcat = np.concatenate([
        g("we1") @ _blockdiag(g("ae1"), H, HID),
        g("we2") @ _blockdiag(g("ae2"), H, HID),
        g("we3") @ _blockdiag(g("ae3"), 1, NC5)], axis=1)

    def wext(w, a_s, a_d, heads, C, cm):
        h = w[:, :]
        out = np.concatenate(
            [h, w @ _blockdiag(a_s, heads, C), w @ _blockdiag(a_d, heads, C)],
            axis=1)
        if cm:
            out[:, :heads * C] = out[:, :heads * C][:, P_CM]
        return out

    w1e = wext(g("w1"), g("as1"), g("ad1"), H, HID, True)
    # layers 2/3 consume c-major h: permute input rows
    w2e = wext(g("w2"), g("as2"), g("ad2"), H, HID, True)[P_CM, :]
    w3e = wext(g("w3"), g("as3"), g("ad3"), 1, NC5, False)[P_CM, :]
    shared = dict(
        w1ext=w1e.astype(BF16),
        w2ext=w2e.astype(BF16),
        w3ext=w3e.astype(BF16),
        ew1=g("ew1").astype(BF16),
        eb1col=np.ascontiguousarray(g("eb1").reshape(HID, 1)),
        w2f=(g("ew2") @ Mcat).astype(BF16),
        cfrow_rep=np.tile(
            np.ascontiguousarray((g("eb2") @ Mcat).reshape(1, 9)), (P, 1)
        ).astype(BF16),
        brep1=np.tile(g("b1")[None, P_CM], (P, 1)),
        brep2=np.tile(g("b2")[None, P_CM], (P, 1)),
        b3rep=np.tile(g("b3")[None, :], (P, 1)),
        al02=np.full((P, 1), 0.2, np.float32),
        idn128=np.eye(P, dtype=np.float32).astype(BF16),
    )
    struct = dict(NSH=NSH, NTILE=NTILE, NSG=NSG, CH=CH, QSH=QSH, QT=QT,
                  TOTBLK=TOTBLK, TOTE=TOTE,
                  btc=btc, boff=boff, lb0=lb0, nblk=nblk, calls=calls,
                  sginfo=sginfo, n_lo=n_lo, new_local=new_local,
                  MAXB=int(btc.max()),
                  MAXNBLK=int(nblk.max()),
                  MAXCALL=max(cb1 - cb0 for cc in calls for cb0, cb1 in cc),
                  MAXSGB=max(sb for _, sb in sginfo))
    return in_maps_core, shared, struct


def _build(s, n_layers=3, dbg_layer=-1):
    import concourse.bass as bass
    import concourse.bacc as bacc
    import concourse.mybir as mybir
    import concourse.tile as tile

    A = mybir.ActivationFunctionType
    OP = mybir.AluOpType
    FP32 = mybir.dt.float32
    BF = mybir.dt.bfloat16
    F8 = mybir.dt.float8e4
    I16 = mybir.dt.int16
    I32 = mybir.dt.int32

    NSH, NTILE, NSG, CH = s["NSH"], s["NTILE"], s["NSG"], s["CH"]
    QSH, QT = s["QSH"], s["QT"]
    TOTBLK, TOTE = s["TOTBLK"], s["TOTE"]
    btc, boff, lb0, nblk = s["btc"], s["boff"], s["lb0"], s["nblk"]
    calls, sginfo = s["calls"], s["sginfo"]
    MAXB, MAXNBLK = s["MAXB"], s["MAXNBLK"]
    MAXCALL, MAXSGB = s["MAXCALL"], s["MAXSGB"]

    nc = bacc.Bacc("TRN2", target_bir_lowering=False, debug=False,
                   enable_asserts=False, num_devices=NCORE, num_swdge_queues=4)

    def dt_in(name, shape, dt):
        return nc.dram_tensor(name, list(shape), dt, kind="ExternalInput").ap()

    gidx_d = dt_in("gidx", (P, TOTE // 16), I16)
    gidx32_d = dt_in("gidx32", (P, TOTBLK), I32)
    ss_d = dt_in("ss", (P, TOTBLK * 2 * P), F8)
    eaT_d = dt_in("eaT", (EF, TOTE), BF)
    xT_d = dt_in("xT", (FIN, NSH), BF)
    w1ext_d = dt_in("w1ext", (FIN, 136), BF)
    w2ext_d = dt_in("w2ext", (HC, 136), BF)
    w3ext_d = dt_in("w3ext", (HC, 7), BF)
    ew1_d = dt_in("ew1", (EF, HID), BF)
    eb1col_d = dt_in("eb1col", (HID, 1), FP32)
    w2f_d = dt_in("w2f", (HID, 9), BF)
    cfrow_rep_d = dt_in("cfrow_rep", (P, 9), BF)
    brep1_d = dt_in("brep1", (P, HC), FP32)
    brep2_d = dt_in("brep2", (P, HC), FP32)
    b3rep_d = dt_in("b3rep", (P, NC5), FP32)
    al02_d = dt_in("al02", (P, 1), FP32)
    idn128_d = dt_in("idn128", (P, P), BF)

    out_d = nc.dram_tensor("out", [NSH, NC5], FP32, kind="ExternalOutput").ap()
    dbg_d = None
    if dbg_layer >= 0:
        dbg_d = nc.dram_tensor("dbg_ht", [P, NSH], FP32,
                               kind="ExternalOutput").ap()

    def mk(base_ap, extra_off, dims):
        return bass.AP(base_ap.tensor, base_ap.offset + extra_off,
                       [base_ap.ap[0]] + dims)

    with tile.TileContext(nc) as tc:
        with tc.tile_pool(name="const", bufs=1) as cst, \
             tc.tile_pool(name="big", bufs=1) as big, \
             tc.tile_pool(name="dram", bufs=1, space="DRAM") as dr:

            def ld_const(ap, shape, dt, nm):
                t = cst.tile(list(shape), dt, name=nm, tag=nm)
                nc.sync.dma_start(out=t[:], in_=ap[:, :])
                return t

            w1ext = ld_const(w1ext_d, (FIN, 136), BF, "w1ext")
            w2ext = ld_const(w2ext_d, (HC, 136), BF, "w2ext")
            w3ext = ld_const(w3ext_d, (HC, 7), BF, "w3ext")
            ew1 = ld_const(ew1_d, (EF, HID), BF, "ew1")
            eb1col = ld_const(eb1col_d, (HID, 1), FP32, "eb1col")
            w2f = ld_const(w2f_d, (HID, 9), BF, "w2f")
            cfrow_rep = ld_const(cfrow_rep_d, (P, 9), BF, "cfrow_rep")
            brep1 = ld_const(brep1_d, (P, HC), FP32, "brep1")
            brep2 = ld_const(brep2_d, (P, HC), FP32, "brep2")
            b3rep = ld_const(b3rep_d, (P, NC5), FP32, "b3rep")
            al02 = ld_const(al02_d, (P, 1), FP32, "al02")
            idn128 = ld_const(idn128_d, (P, P), BF, "idn128")
            # small epilogue constants built on-chip
            epsH = cst.tile([P, H], FP32, name="epsH", tag="epsH")
            nc.vector.memset(epsH[:], 1e-16)
            ones_hc = cst.tile([P, HC], FP32, name="ones_hc", tag="ones_hc")
            nc.vector.memset(ones_hc[:], 1.0)
            zero_hc = cst.tile([P, HC], FP32, name="zero_hc", tag="zero_hc")
            nc.vector.memset(zero_hc[:], 0.0)
            eps1 = cst.tile([P, 1], FP32, name="eps1", tag="eps1")
            nc.vector.memset(eps1[:], 1e-16)
            zero1 = cst.tile([P, 1], FP32, name="zero1", tag="zero1")
            nc.vector.memset(zero1[:], 0.0)

            ht = big.tile([P, NSH], BF)
            adsb = [big.tile([P, NTILE * H], BF, name=f"adsb{i}", tag=f"adsb{i}")
                    for i in range(3)]
            AEC = dr.tile([P, TOTBLK * 9], BF, name="aecd")

            Tsh = [[dr.tile([QSH, ROW if l < 2 else ROW3], BF,
                            name=f"tsh{l}q{q}") for q in range(NCH)]
                   for l in range(3)]
            Tf = [[dr.tile([CH, ROW if l < 2 else ROW3], BF,
                           name=f"tf{l}q{q}", addr_space="Shared")
                   for q in range(NCH)] for l in range(3)]

            xt_cm = tc.tile_pool(name="xtp", bufs=1)
            xt_pool = xt_cm.__enter__()
            xt = xt_pool.tile([FIN, NSH], BF, name="xt")
            nc.sync.dma_start(out=xt[:], in_=xT_d[:, :])

            def projection(lay, pps, stg_p):
                K = FIN if lay == 0 else HC
                lhs = xt if lay == 0 else ht
                wx = (w1ext, w2ext, w3ext)[lay]
                ncol = 7 if lay == 2 else 136
                rw = ROW3 if lay == 2 else ROW
                adw = 1 if lay == 2 else H
                adoff = 6 if lay == 2 else 132
                for tp in range(NTILE):
                    pp = pps.tile([P, 136], FP32, space="PSUM", tag="proj",
                                  name="proj")
                    nc.tensor.matmul(pp[:, :ncol],
                                     lhsT=lhs[:K, tp * P:(tp + 1) * P],
                                     rhs=wx[:], start=True, stop=True)
                    st_t = stg_p.tile([P, ROW], BF, tag="tstg", name="tstg")
                    nc.vector.tensor_copy(out=st_t[:, :ncol], in_=pp[:, :ncol])
                    nc.vector.tensor_copy(
                        out=adsb[lay][:, tp * adw:(tp + 1) * adw],
                        in_=pp[:, adoff:adoff + adw])
                    q, tq = tp // QT, tp % QT
                    nc.sync.dma_start(
                        out=Tsh[lay][q][tq * P:(tq + 1) * P, :],
                        in_=st_t[:, :rw])
                    if tq == QT - 1:
                        nc.gpsimd.collective_compute(
                            "AllGather", OP.bypass,
                            replica_groups=[list(range(NCORE))],
                            ins=[Tsh[lay][q].opt()], outs=[Tf[lay][q].opt()])

            # ---------------- layer-1 projection (before encoder so the
            # AllGather + gathers start as early as possible) ---------------
            with tc.tile_pool(name="p0ps", bufs=2, space="PSUM") as p0ps, \
                 tc.tile_pool(name="p0stg", bufs=3) as p0stg:
                projection(0, p0ps, p0stg)
            xt_cm.__exit__(None, None, None)

            # ---------------- layers (encoder interleaved into L1) --------
            with tc.tile_pool(name="mps", bufs=2, space="PSUM") as pps, \
                 tc.tile_pool(name="ade_ps", bufs=1, space="PSUM") as pade, \
                 tc.tile_pool(name="agg_ps", bufs=2, space="PSUM") as pagg, \
                 tc.tile_pool(name="tr_ps", bufs=1, space="PSUM") as ptr, \
                 tc.tile_pool(name="enc_sb", bufs=4) as esb, \
                 tc.tile_pool(name="enc_ps", bufs=1, space="PSUM") as eps, \
                 tc.tile_pool(name="enc_ps2", bufs=1, space="PSUM") as eps2, \
                 tc.tile_pool(name="stgp", bufs=3) as stg_p, \
                 tc.tile_pool(name="gp", bufs=5 if SGT <= 2 else 2) as gp, \
                 tc.tile_pool(name="stp", bufs=2) as stp, \
                 tc.tile_pool(name="zp", bufs=4) as zp, \
                 tc.tile_pool(name="ep", bufs=4) as ep, \
                 tc.tile_pool(name="ip", bufs=10) as ip, \
                 tc.tile_pool(name="aep", bufs=3) as aep:

                EG = 16
                enc_next = [0]

                def emit_enc(ngroups):
                    for _ in range(ngroups):
                        eg0 = enc_next[0]
                        if eg0 >= TOTBLK:
                            return
                        enc_next[0] = eg0 + EG
                        nb = min(EG, TOTBLK - eg0)
                        ne = nb * P
                        ea_t = esb.tile([EF, EG * P], BF, tag="ea", name="ea")
                        nc.sync.dma_start(out=ea_t[:, :ne],
                                          in_=eaT_d[:, eg0 * P:eg0 * P + ne])
                        aest = esb.tile([P, EG * 9], BF, tag="aest", name="aest")
                        for q0 in range(0, ne, 512):
                            qn = min(512, ne - q0)
                            nsub = qn // P
                            hidp = eps.tile([HID, 512], FP32, space="PSUM",
                                            tag="hid", name="hid")
                            nc.tensor.matmul(hidp[:, :qn], lhsT=ew1[:],
                                             rhs=ea_t[:, q0:q0 + qn],
                                             start=True, stop=True)
                            hids = esb.tile([HID, 512], BF, tag="hids",
                                            name="hids")
                            nc.scalar.activation(hids[:, :qn], hidp[:, :qn],
                                                 A.Relu, bias=eb1col[:],
                                                 scale=1.0)
                            pae = eps2.tile([P, 36], FP32, space="PSUM",
                                            tag="pae", name="pae")
                            for sb_ in range(nsub):
                                sl = pae[:, sb_ * 9:sb_ * 9 + 9]
                                nc.tensor.matmul(
                                    sl, lhsT=hids[:, sb_ * P:(sb_ + 1) * P],
                                    rhs=w2f[:], start=True, stop=True)
                            col = (q0 // P) * 9
                            nc.vector.tensor_tensor(
                                out=aest[:, col:col + nsub * 9],
                                in0=pae[:, :nsub * 9],
                                in1=mk(cfrow_rep[:], 0, [[0, nsub], [1, 9]]),
                                op=OP.add)
                        nc.sync.dma_start(out=AEC[:, eg0 * 9:(eg0 + nb) * 9],
                                          in_=aest[:, :nb * 9])

                def attention(lay, pre=None):
                    rw = ROW3 if lay == 2 else ROW
                    vw = 6 if lay == 2 else 132
                    aw = 1 if lay == 2 else H
                    acol = NC5 if lay == 2 else HC
                    CC = NC5 if lay == 2 else HID  # features per head
                    aecol = (0, 4, 8)[lay]
                    brep = (brep1, brep2, None)[lay]
                    for sg in range(NSG):
                        if pre is not None:
                            pre()
                        sgb0, sgblk = sginfo[sg]
                        if sgblk == 0:
                            continue
                        aec_t = aep.tile([P, MAXSGB * 9], BF, tag="aec",
                                         name="aec")
                        nc.sync.dma_start(out=aec_t[:, :sgblk * 9],
                                          in_=AEC[:, sgb0 * 9:(sgb0 + sgblk) * 9])
                        ss_sg = stp.tile([P, MAXSGB * 2 * P], F8, tag="ss",
                                         name="ss")
                        nc.scalar.dma_start(
                            out=ss_sg[:, :sgblk * 2 * P],
                            in_=ss_d[:, sgb0 * 2 * P:(sgb0 + sgblk) * 2 * P])
                        g_t = {}
                        for c in range(NCH):
                            cb0, cb1 = calls[sg][c]
                            nn = cb1 - cb0
                            if nn == 0:
                                continue
                            gt = gp.tile([P, MAXCALL, rw], BF, tag=f"g{c}")
                            if c in IND_CH:
                                it32 = ip.tile([P, MAXCALL], I32, tag="idx32",
                                               name="idx32")
                                nc.sync.dma_start(out=it32[:, :nn],
                                                  in_=gidx32_d[:, cb0:cb1])
                                nc.gpsimd.indirect_dma_start(
                                    out=gt[:, :nn, :], out_offset=None,
                                    in_=Tf[lay][c][:, :],
                                    in_offset=bass.IndirectOffsetOnAxis(
                                        ap=it32[:, :nn], axis=0))
                            else:
                                it = ip.tile([P, MAXCALL * 8], I16, tag="idx",
                                             name="idx")
                                nc.sync.dma_start(out=it[:, :nn * 8],
                                                  in_=gidx_d[:, cb0 * 8:cb1 * 8])
                                nc.gpsimd.dma_gather(
                                    out_ap=gt[:, :nn, :],
                                    in_ap=Tf[lay][c][:, :],
                                    idxs_ap=it[:, :nn * 8],
                                    num_idxs=nn * P, num_idxs_reg=nn * P,
                                    elem_size=rw, single_packet=False,
                                    queue_num=c)
                            g_t[c] = gt
                        for t in range(sg * SGT, (sg + 1) * SGT):
                            nb = int(nblk[t])
                            aggp = pagg.tile([P, 132], FP32, space="PSUM",
                                             tag="agg")
                            if nb == 0:
                                nc.vector.memset(aggp[:, :vw], 0.0)
                            else:
                                adt_sl = adsb[lay][:, t * aw:(t + 1) * aw]
                                adep = pade.tile([P, MAXNBLK * H], FP32,
                                                 space="PSUM", tag="ade")
                                z1 = zp.tile([P, MAXNBLK * H], FP32, tag="z1",
                                             name="z1")
                                for c in range(NCH):
                                    b = int(btc[t, c])
                                    if b == 0:
                                        continue
                                    bo = int(boff[t, c])
                                    lb = int(lb0[t, c])
                                    s0 = bo - calls[sg][c][0]
                                    sb = bo - sgb0
                                    for bi in range(b):
                                        nc.tensor.matmul(
                                            adep[:, (lb + bi) * aw:
                                                 (lb + bi + 1) * aw],
                                            lhsT=ss_sg[:, (sb + bi) * 2 * P:
                                                       (sb + bi) * 2 * P + P],
                                            rhs=adt_sl,
                                            start=True, stop=True)
                                    gb = g_t[c][:]          # [P, MAXCALL, ROW]
                                    gstep = gb.ap[1][0]     # ROW stride
                                    z1sl = mk(z1[:], lb * aw,
                                              [[aw, b], [1, aw]])
                                    as_ap = mk(gb, s0 * gstep + acol,
                                               [[gstep, b], [1, aw]])
                                    ae_ap = mk(aec_t[:],
                                               ((bo - sgb0) * 9 + aecol),
                                               [[9, b], [1, aw]])
                                    nc.vector.tensor_tensor(
                                        out=z1sl, in0=as_ap, in1=ae_ap, op=OP.add)
                                zz = zp.tile([P, MAXNBLK * H], FP32, tag="zz",
                                             name="zz")
                                nc.vector.tensor_tensor(
                                    out=zz[:, :nb * aw], in0=z1[:, :nb * aw],
                                    in1=adep[:, :nb * aw], op=OP.add)
                                zpre = zp.tile([P, MAXNBLK * H], FP32,
                                               tag="zpre", name="zpre")
                                nc.scalar.activation(zpre[:, :nb * aw],
                                                     zz[:, :nb * aw], A.Prelu,
                                                     bias=0.0, scale=1.0,
                                                     alpha=al02[:])
                                mmi = 0
                                for c in range(NCH):
                                    b = int(btc[t, c])
                                    if b == 0:
                                        continue
                                    bo = int(boff[t, c])
                                    lb = int(lb0[t, c])
                                    s0 = bo - calls[sg][c][0]
                                    sb = bo - sgb0
                                    gb = g_t[c][:]
                                    gstep = gb.ap[1][0]
                                    ex_ap = mk(gb, s0 * gstep + acol,
                                               [[gstep, b], [1, aw]])
                                    nc.scalar.activation(
                                        ex_ap, mk(zpre[:], lb * aw,
                                                  [[aw, b], [1, aw]]),
                                        A.Exp, bias=0.0, scale=1.0)
                                    if lay < 2:
                                        # c-major: inner dim over H contiguous
                                        v_in = mk(gb, s0 * gstep,
                                                  [[gstep, b], [aw, CC],
                                                   [1, aw]])
                                        a_in = mk(gb, s0 * gstep + acol,
                                                  [[gstep, b], [0, CC],
                                                   [1, aw]])
                                    else:
                                        v_in = mk(gb, s0 * gstep,
                                                  [[gstep, b], [CC, aw],
                                                   [1, CC]])
                                        a_in = mk(gb, s0 * gstep + acol,
                                                  [[gstep, b], [1, aw],
                                                   [0, CC]])
                                    nc.vector.tensor_tensor(
                                        out=v_in, in0=v_in, in1=a_in, op=OP.mult)
                                    for bi in range(b):
                                        nc.tensor.matmul(
                                            aggp[:, :vw],
                                            lhsT=ss_sg[:, (sb + bi) * 2 * P + P:
                                                       (sb + bi + 1) * 2 * P],
                                            rhs=mk(gb, (s0 + bi) * gstep,
                                                   [[1, vw]]),
                                            start=(mmi == 0),
                                            stop=(mmi == nb - 1))
                                        mmi += 1
                            # epilogue: move [agg|den] to SBUF, free PSUM fast
                            agg_s = ep.tile([P, 132], FP32, tag="aggs",
                                            name="aggs")
                            nc.vector.tensor_copy(out=agg_s[:, :vw],
                                                  in_=aggp[:, :vw])
                            if lay < 2:
                                t1 = ep.tile([P, H], FP32, tag="t1", name="t1")
                                nc.vector.tensor_tensor(
                                    out=t1[:], in0=agg_s[:, HC:HC + H],
                                    in1=epsH[:], op=OP.add)
                                rden = ep.tile([P, H], FP32, tag="rden",
                                               name="rden")
                                nc.vector.reciprocal(out=rden[:], in_=t1[:])
                                xn = ep.tile([P, HC], FP32, tag="xn", name="xn")
                                # c-major: den broadcast inner-contiguous
                                nc.vector.tensor_tensor(
                                    out=xn[:], in0=agg_s[:, :HC],
                                    in1=mk(rden[:], 0, [[0, HID], [1, H]]),
                                    op=OP.mult)
                                xb = ep.tile([P, HC], FP32, tag="xb", name="xb")
                                nc.vector.tensor_tensor(out=xb[:], in0=xn[:],
                                                        in1=brep[:], op=OP.add)
                                e1 = ep.tile([P, HC], FP32, tag="e1", name="e1")
                                nc.scalar.activation(e1[:], xb[:], A.Exp,
                                                     bias=0.0, scale=1.0)
                                em1 = ep.tile([P, HC], FP32, tag="em1",
                                              name="em1")
                                nc.vector.tensor_tensor(out=em1[:], in0=e1[:],
                                                        in1=ones_hc[:],
                                                        op=OP.subtract)
                                t2 = ep.tile([P, HC], FP32, tag="t2", name="t2")
                                nc.vector.tensor_tensor(out=t2[:], in0=em1[:],
                                                        in1=zero_hc[:],
                                                        op=OP.min)
                                r1 = ep.tile([P, HC], FP32, tag="r1", name="r1")
                                nc.scalar.activation(r1[:], xb[:], A.Relu,
                                                     bias=0.0, scale=1.0)
                                hn = ep.tile([P, HC], BF, tag="hn", name="hn")
                                nc.vector.tensor_tensor(out=hn[:], in0=t2[:],
                                                        in1=r1[:], op=OP.add)
                                htp = ptr.tile([P, P], BF, space="PSUM",
                                               tag="htp")
                                nc.tensor.transpose(out=htp[:], in_=hn[:],
                                                    identity=idn128[:])
                                nc.vector.tensor_copy(
                                    out=ht[:, t * P:(t + 1) * P], in_=htp[:])
                            else:
                                t1 = ep.tile([P, 1], FP32, tag="t1", name="t1")
                                nc.vector.tensor_tensor(
                                    out=t1[:], in0=agg_s[:, NC5:NC5 + 1],
                                    in1=eps1[:], op=OP.add)
                                rden = ep.tile([P, 1], FP32, tag="rden",
                                               name="rden")
                                nc.vector.reciprocal(out=rden[:], in_=t1[:])
                                x5 = ep.tile([P, NC5], FP32, tag="xn", name="xn")
                                nc.vector.tensor_tensor(
                                    out=x5[:], in0=agg_s[:, :NC5],
                                    in1=mk(rden[:], 0, [[0, NC5]]),
                                    op=OP.mult)
                                xb5 = ep.tile([P, NC5], FP32, tag="xb",
                                              name="xb")
                                nc.vector.tensor_tensor(out=xb5[:], in0=x5[:],
                                                        in1=b3rep[:], op=OP.add)
                                m1 = ep.tile([P, 1], FP32, tag="m1", name="m1")
                                nc.vector.reduce_max(out=m1[:], in_=xb5[:],
                                                     axis=mybir.AxisListType.X)
                                negm = ep.tile([P, 1], FP32, tag="negm",
                                               name="negm")
                                nc.vector.tensor_tensor(
                                    out=negm[:], in0=zero1[:], in1=m1[:],
                                    op=OP.subtract)
                                e5 = ep.tile([P, NC5], FP32, tag="e1", name="e1")
                                nc.scalar.activation(e5[:], xb5[:], A.Exp,
                                                     bias=negm[:], scale=1.0)
                                ssum = ep.tile([P, 1], FP32, tag="ssum",
                                               name="ssum")
                                nc.vector.reduce_sum(out=ssum[:], in_=e5[:],
                                                     axis=mybir.AxisListType.X)
                                lns = ep.tile([P, 1], FP32, tag="lns",
                                              name="lns")
                                nc.scalar.activation(lns[:], ssum[:], A.Ln,
                                                     bias=0.0, scale=1.0)
                                mls = ep.tile([P, 1], FP32, tag="mls",
                                              name="mls")
                                nc.vector.tensor_tensor(out=mls[:], in0=m1[:],
                                                        in1=lns[:], op=OP.add)
                                o5 = ep.tile([P, NC5], FP32, tag="o5",
                                             name="o5")
                                nc.vector.tensor_tensor(
                                    out=o5[:], in0=xb5[:],
                                    in1=mk(mls[:], 0, [[0, NC5]]),
                                    op=OP.subtract)
                                nc.sync.dma_start(
                                    out=out_d[t * P:(t + 1) * P, :], in_=o5[:])

                for lay in range(n_layers):
                    if lay > 0:
                        projection(lay, pps, stg_p)
                    attention(lay, pre=(lambda: emit_enc(3))
                              if lay == 0 else None)
                    if lay == 0:
                        emit_enc(TOTBLK)  # drain any leftover encoder groups
                    if dbg_layer == lay and dbg_d is not None and lay < 2:
                        nc.gpsimd.dma_start(out=dbg_d[:, :], in_=ht[:])
    nc.compile()
    return nc


def kernel(**inputs):
    from concourse import bass_utils
    in_maps_core, shared, struct = _prep(inputs)
    n_layers = int(os.environ.get("GAT_LAYERS", "3"))
    dbg_layer = int(os.environ.get("GAT_DEBUG_LAYER", "-1"))
    nc = _build(struct, n_layers=n_layers, dbg_layer=dbg_layer)
    in_maps = []
    for k in range(NCORE):
        m = dict(in_maps_core[k])
        m.update(shared)
        in_maps.append(m)
    trace = os.environ.get("GAT_TRACE", "0") == "1"
    res = bass_utils.run_bass_kernel_spmd(
        nc, in_maps, core_ids=list(range(NCORE)), trace=trace)
    kernel.last_result = res
    kernel.last_struct = struct
    n_lo = struct["n_lo"]
    nl = struct["new_local"]
    out = np.zeros((N, NC5), np.float32)
    for k in range(NCORE):
        ids = np.arange(n_lo[k], n_lo[k + 1])
        out[ids] = res.results[k]["out"][nl[ids]]
    return out
